# revision 30
# baseline (speedup 1.0000x reference)
"""Causal multi-head attention (B=4, S=2048, H=16, Dh=64) on 8 TRN2 NeuronCores.

Sharding: core c = (batch b=c//2, parity g=c%2). Each core handles the
interleaved 64-row query blocks {64*(2i+g) : i=0..15} of its batch (zig-zag
causal load balancing -> identical SPMD program + equal work on all cores),
with the full K/V of that batch. No collectives needed; host concatenates.

Compute dtype: bf16 matmul inputs, fp32 PSUM accumulation, fp32 output.

Device pipeline per core:
  V = token-major proj w/ interleaved ones column (softmax denominator trick),
  KT = feature-major K proj, then per head-pair: QT proj (WQ prescaled 1/8),
  scores S^T[k,q] (two heads row-tiled concurrently) -> exp (no max-sub;
  logits ~N(0,1)) -> causal mask multiply on diagonal 64-col blocks ->
  AV accumulate (denominator lands in PSUM row 64) -> normalize via
  reciprocal + PE ones-broadcast -> WO projection + bias.
"""

import numpy as np
import ml_dtypes

B = 4
S = 2048
D = 1024
H = 16
DH = 64
NCORES = 8
NSLOT = 16  # 64-row query blocks per core
QB = 64  # query block rows
P = 128

_CACHE = {}


def _build():
    import concourse.mybir as mybir
    import concourse.tile as tile
    from concourse import bacc

    dt = mybir.dt
    BF = dt.bfloat16
    F32 = dt.float32
    AF = mybir.ActivationFunctionType

    nc = bacc.Bacc("TRN2", target_bir_lowering=False, debug=False,
                   num_devices=NCORES)

    qT_d = nc.declare_dram_parameter("qT", [D, 1024], BF, isOutput=False)
    kT_d = nc.declare_dram_parameter("kT", [D, S], BF, isOutput=False)
    vT_d = nc.declare_dram_parameter("vT", [D, S], BF, isOutput=False)
    wq_d = nc.declare_dram_parameter("wq", [D, D], BF, isOutput=False)
    wk_d = nc.declare_dram_parameter("wk", [D, D], BF, isOutput=False)
    wv_d = nc.declare_dram_parameter("wv", [D, D], BF, isOutput=False)
    wo_d = nc.declare_dram_parameter("wo", [D, D], BF, isOutput=False)
    bq_d = nc.declare_dram_parameter("bq", [P, 8], F32, isOutput=False)
    bk_d = nc.declare_dram_parameter("bk", [P, 8], F32, isOutput=False)
    bvo_d = nc.declare_dram_parameter("bvo", [1, 2 * D], BF, isOutput=False)
    mask_d = nc.declare_dram_parameter("masks", [P, 2 * QB], BF, isOutput=False)
    out_d = nc.declare_dram_parameter("out", [1024, D], F32, isOutput=True)

    with tile.TileContext(nc) as tc:
        with tc.tile_pool(name="persist", bufs=1) as pp:
            # ---- persistent SBUF tensors (~101 KB/partition) ----
            # V augmented: per token-tile t, head h at cols [65h, 65h+64) plus
            # ones at col 65h+64 (softmax denominator via matmul).
            # per head: cols 0..63 = V, col 64 = ones (denominator row 64)
            Vaug_sb = [pp.tile([P, 65 * H], BF, name=f"Vaug{t}", tag=f"Vaug{t}")
                       for t in range(S // P)]
            OT_sb = [pp.tile([P, 1024], BF, name=f"OTsb{f}", tag=f"OTsb{f}")
                     for f in range(8)]
            ones_sb = pp.tile([1, P], BF, name="ones", tag="ones")
            onesf_sb = pp.tile([1, P], F32, name="onesf", tag="onesf")
            mask_sb = pp.tile([P, 2 * QB], BF, name="masksb", tag="masksb")
            bq_sb = pp.tile([P, 8], F32, name="bqsb", tag="bqsb")
            bk_sb = pp.tile([P, 8], F32, name="bksb", tag="bksb")
            bvo_sb = pp.tile([1, 2 * D], BF, name="bvosb", tag="bvosb")

            nc.vector.memset(ones_sb[:], 1.0)
            nc.vector.memset(onesf_sb[:], 1.0)
            nc.sync.dma_start(mask_sb[:], mask_d[:, :])
            nc.sync.dma_start(bq_sb[:], bq_d[:, :])
            nc.sync.dma_start(bk_sb[:], bk_d[:, :])
            nc.sync.dma_start(bvo_sb[:], bvo_d[:, :])

            # Attention-stage pools open BEFORE the V phase so their SBUF is
            # disjoint from vstage: the kT/qT input DMAs then stream at t=0
            # instead of WAR-waiting for V-proj to release its buffers.
            astack = (
                tc.tile_pool(name="astage", bufs=1),
                tc.tile_pool(name="ktpool", bufs=2),
                tc.tile_pool(name="qtpool", bufs=2),
                tc.tile_pool(name="ppool", bufs=8),
                tc.tile_pool(name="npool", bufs=1),
                tc.tile_pool(name="vstage", bufs=1),
            )
            asp, ktp, qtp, ppool, npool, vsp = [
                p.__enter__() for p in astack]

            wv_sb = [vsp.tile([P, D], BF, name=f"wv{i}", tag=f"wv{i}")
                     for i in range(8)]
            vT_sb = [vsp.tile([P, S], BF, name=f"vT{i}", tag=f"vT{i}")
                     for i in range(8)]
            for i in range(8):
                nc.sync.dma_start(wv_sb[i][:], wv_d[P * i:P * i + P, :])
                nc.sync.dma_start(vT_sb[i][:], vT_d[P * i:P * i + P, :])

            # ========= interleaved: KT proj + QT proj + attention ==========
            # Projection matmul groups for head-pair hp+1 are woven between
            # attention blocks of hp as dependency-free PE filler, so the
            # TensorEngine never idles long enough for HAM to re-throttle.
            F32R = dt.float32r
            with (
                # PSUM: "spair" [P,1024] (2 banks) x 2 + "oe"/"oo" [65,512]
                # (1 bank) x 1 each + "sps" [P,512] x 2 = 8 banks total.
                tc.tile_pool(name="spsum", bufs=1, space="PSUM") as sps,
                tc.tile_pool(name="opsum", bufs=1, space="PSUM") as ops,
            ):
                wk_sb = [asp.tile([P, D], BF, name=f"wk{i}", tag=f"wk{i}")
                         for i in range(8)]
                kT_sb = [asp.tile([P, S], BF, name=f"kT{i}", tag=f"kT{i}")
                         for i in range(8)]
                wq_sb = [asp.tile([P, D], BF, name=f"wq{i}", tag=f"wq{i}")
                         for i in range(8)]
                qT_sb = [asp.tile([P, 1024], BF, name=f"qT{i}", tag=f"qT{i}")
                         for i in range(8)]
                for i in range(8):
                    nc.sync.dma_start(wk_sb[i][:], wk_d[P * i:P * i + P, :])
                    nc.sync.dma_start(kT_sb[i][:], kT_d[P * i:P * i + P, :])
                for i in range(8):
                    nc.sync.dma_start(wq_sb[i][:], wq_d[P * i:P * i + P, :])
                    nc.sync.dma_start(qT_sb[i][:], qT_d[P * i:P * i + P, :])

                def _v_group(t, half):
                    if half == 0:
                        nc.vector.memset(Vaug_sb[t][:], 1.0)
                    ps = sps.tile([P, 512], F32, name="sps", tag="sps")
                    for fi in range(8):
                        nc.tensor.matmul(
                            ps[:],
                            lhsT=vT_sb[fi][:, P * t:P * t + P],
                            rhs=wv_sb[fi][:, 512 * half:512 * half + 512],
                            start=(fi == 0), stop=False)
                    nc.tensor.matmul(
                        ps[:],
                        lhsT=ones_sb[0:1, 0:P],
                        rhs=bvo_sb[0:1, 512 * half:512 * half + 512],
                        start=False, stop=True)
                    src_ap = ps[:].rearrange("p (h w) -> p h w", w=DH)
                    dst = Vaug_sb[t][:].rearrange(
                        "p (h w) -> p h w",
                        w=65)[:, 8 * half:8 * half + 8, 0:DH]
                    nc.vector.tensor_copy(dst, src_ap)

                KT_ts = {}
                QT_ts = {}

                def _alloc_proj(php):
                    KT_ts[php] = ktp.tile([P, S], BF, name="KTrot",
                                          tag="KTrot")
                    QT_ts[php] = qtp.tile([P, 1024], BF, name="QTrot",
                                          tag="QTrot")

                def _kt_group(php, nck):
                    ps = sps.tile([P, 512], F32, name="sps", tag="sps")
                    for fi in range(8):
                        nc.tensor.matmul(
                            ps[:, 0:512],
                            lhsT=wk_sb[fi][:, P * php:P * php + P],
                            rhs=kT_sb[fi][:, 512 * nck:512 * nck + 512],
                            start=(fi == 0), stop=(fi == 7))
                    nc.vector.tensor_scalar(
                        out=KT_ts[php][:, 512 * nck:512 * nck + 512],
                        in0=ps[:, 0:512], scalar1=bk_sb[:, php:php + 1],
                        scalar2=None, op0=mybir.AluOpType.add)

                def _qt_group(php, nck):
                    ps = sps.tile([P, 512], F32, name="sps", tag="sps")
                    for fi in range(8):
                        nc.tensor.matmul(
                            ps[:, 0:512],
                            lhsT=wq_sb[fi][:, P * php:P * php + P],
                            rhs=qT_sb[fi][:, 512 * nck:512 * nck + 512],
                            start=(fi == 0), stop=(fi == 7))
                    nc.vector.tensor_scalar(
                        out=QT_ts[php][:, 512 * nck:512 * nck + 512],
                        in0=ps[:, 0:512], scalar1=bq_sb[:, php:php + 1],
                        scalar2=None, op0=mybir.AluOpType.add)

                pend = []  # (hp, o_psum, head_parity) awaiting normalization

                def _normalize():
                    nhp, nqh, o_ps, h = pend.pop(0)
                    d_sb = npool.tile([1, 512], F32, name="dsb", tag="dsb")
                    nc.vector.tensor_copy(d_sb[:], o_ps[64:65, 0:512])
                    r_sb = npool.tile([1, 512], F32, name="rsb", tag="rsb")
                    with nc.allow_low_precision(
                            reason="recip feeds bf16 output"):
                        nc.vector.reciprocal_approx_fast(r_sb[:], d_sb[:])
                    rb_sb = npool.tile([1, 512], BF, name="rbsb", tag="rbsb")
                    nc.vector.tensor_copy(rb_sb[:], r_sb[:])
                    b_ps = sps.tile([P, 512], F32, name="sps", tag="sps")
                    nc.tensor.matmul(
                        b_ps[0:64, 0:512],
                        lhsT=ones_sb[0:1, 0:64],
                        rhs=rb_sb[0:1, 0:512],
                        start=True, stop=True)
                    b_sb = npool.tile([64, 512], BF, name="bsb", tag="bsb")
                    nc.vector.tensor_copy(b_sb[:], b_ps[0:64, 0:512])
                    nc.vector.tensor_mul(
                        OT_sb[nhp][64 * h:64 * h + 64,
                                   512 * nqh:512 * nqh + 512],
                        o_ps[0:64, 0:512], b_sb[:])

                for t in range(8):
                    for half in range(2):
                        _v_group(t, half)
                _alloc_proj(0)
                for nck in range(4):
                    _kt_group(0, nck)
                for nck in range(2):
                    _qt_group(0, nck)

                for hp in range(8):  # head pair = fo-tile index
                    KT_t = KT_ts[hp]
                    QT_t = QT_ts[hp]
                    fillers = []
                    if hp == 0:
                        fillers += [
                            (lambda t=t, hf=hf: _v_group(t, hf))
                            for t in range(8, 16) for hf in range(2)]
                    if hp + 1 < 8:
                        _alloc_proj(hp + 1)
                        fillers += [
                            (lambda n=n: _kt_group(hp + 1, n))
                            for n in range(4)]
                        fillers += [
                            (lambda n=n: _qt_group(hp + 1, n))
                            for n in range(2)]

                    # --- attention for head pair hp, one q-half at a time ---
                    for qh in range(2):
                        q0 = 512 * qh
                        # drain pending normalizations before reusing o psum
                        while pend:
                            _normalize()
                        o_e = ops.tile([65, 512], F32, name="oe", tag="oe")
                        o_o = ops.tile([65, 512], F32, name="oo", tag="oo")
                        kts = [kt for kt in range(NSLOT)
                               if QB * kt < q0 + 512]
                        nkts = len(kts)
                        for ki, kt in enumerate(kts):
                            c0 = max(QB * kt, q0)
                            c1 = q0 + 512
                            w = c1 - c0
                            # per-head 1-bank score tiles (bufs=5) give
                            # the PE a deeper dependency lookahead; adjacent
                            # emission still packs the K=64 pair into
                            # disjoint PE row groups
                            sp2 = [sps.tile([P, 512], F32, name="shead",
                                            tag="shead", bufs=5)
                                   for _ in range(2)]
                            for h in range(2):
                                nc.tensor.matmul(
                                    sp2[h][:, 0:w],
                                    lhsT=KT_t[64 * h:64 * h + 64,
                                              P * kt:P * kt + P],
                                    rhs=QT_t[64 * h:64 * h + 64, c0:c1],
                                    start=True, stop=True)
                            pt2 = [ppool.tile([P, 512], BF, name="pt",
                                              tag="pt") for _ in range(2)]
                            for h in range(2):
                                nc.scalar.activation(pt2[h][:, 0:w],
                                                     sp2[h][:, 0:w], AF.Exp)
                            if c0 == QB * kt:  # diagonal 64-col block
                                for h in range(2):
                                    nc.vector.tensor_mul(
                                        pt2[h][:, 0:QB], pt2[h][:, 0:QB],
                                        mask_sb[:, QB * h:QB * h + QB])
                            for h in range(2):
                                o_ps = o_e if h == 0 else o_o
                                nc.tensor.matmul(
                                    o_ps[0:65, c0 - q0:c1 - q0],
                                    lhsT=Vaug_sb[kt][:, 65 * (2 * hp + h):
                                                     65 * (2 * hp + h) + 65],
                                    rhs=pt2[h][:, 0:w],
                                    start=(ki == 0), stop=(ki == nkts - 1),
                                    skip_group_check=True)
                            if fillers and (hp == 0 or ki % 4 == 3):
                                fillers.pop(0)()
                        pend.append((hp, qh, o_e, 0))
                        pend.append((hp, qh, o_o, 1))
                    while fillers:
                        fillers.pop(0)()
                while pend:
                    _normalize()

            for p in reversed(astack):
                p.__exit__(None, None, None)

            # ======================= WO projection =========================
            with (
                tc.tile_pool(name="wops", bufs=4, space="PSUM") as wps,
                tc.tile_pool(name="wosb", bufs=1) as wop,
                tc.tile_pool(name="stage", bufs=3) as stp,
            ):
                wo_sb = [wop.tile([P, D], BF, name=f"wo{i}", tag=f"wo{i}")
                         for i in range(8)]
                for i in range(8):
                    nc.sync.dma_start(wo_sb[i][:], wo_d[P * i:P * i + P, :])
                for qt in range(8):
                    st = stp.tile([P, D], F32, name="st", tag="st")
                    for half in range(2):
                        ps = wps.tile([P, 512], F32, name="wps", tag="wps")
                        for f in range(8):
                            nc.tensor.matmul(
                                ps[:],
                                lhsT=OT_sb[f][:, P * qt:P * qt + P],
                                rhs=wo_sb[f][:, 512 * half:512 * half + 512],
                                start=(f == 0), stop=False)
                        nc.tensor.matmul(
                            ps[:],
                            lhsT=ones_sb[0:1, 0:P],
                            rhs=bvo_sb[0:1, D + 512 * half:D + 512 * half + 512],
                            start=False, stop=True)
                        nc.scalar.copy(st[:, 512 * half:512 * half + 512],
                                       ps[:])
                    nc.sync.dma_start(out_d[P * qt:P * qt + P, :], st[:])

    nc.compile()
    return nc


def _get_nc():
    if "nc" not in _CACHE:
        _CACHE["nc"] = _build()
    return _CACHE["nc"]


def _perm(g):
    # local row 64*i+f  <->  global row 64*(2*i+g)+f
    return np.concatenate(
        [QB * (2 * i + g) + np.arange(QB) for i in range(NSLOT)])


def _make_in_maps(q, k, v, WQ, WQ_bias, WK, WK_bias, WV, WV_bias, WO, WO_bias):
    bf = ml_dtypes.bfloat16
    scale = np.float32(1.0 / np.sqrt(DH))
    wq = np.ascontiguousarray(WQ * scale).astype(bf)
    wk = np.ascontiguousarray(WK).astype(bf)
    wv = np.ascontiguousarray(WV).astype(bf)
    wo = np.ascontiguousarray(WO).astype(bf)
    bq = np.ascontiguousarray((WQ_bias * scale).reshape(8, P).T).astype(
        np.float32)
    bk = np.ascontiguousarray(WK_bias.reshape(8, P).T).astype(np.float32)
    bvo = np.ascontiguousarray(
        np.concatenate([WV_bias, WO_bias])[None, :]).astype(bf)

    kT = [np.ascontiguousarray(k[b].T).astype(bf) for b in range(B)]
    vT = [np.ascontiguousarray(v[b].T).astype(bf) for b in range(B)]

    in_maps = []
    for c in range(NCORES):
        b, g = c // 2, c % 2
        perm = _perm(g)
        qT = np.ascontiguousarray(q[b][perm].T).astype(bf)
        # mask for the diagonal 64-col block: [p, 64h+f] = (f >= p - 64*g)
        pgrid = np.arange(P)[:, None]
        fgrid = np.arange(QB)[None, :]
        m = (fgrid >= pgrid - QB * g).astype(np.float32)
        masks = np.ascontiguousarray(np.concatenate([m, m], axis=1)).astype(bf)
        in_maps.append({
            "qT": qT, "kT": kT[b], "vT": vT[b],
            "wq": wq, "wk": wk, "wv": wv, "wo": wo,
            "bq": bq, "bk": bk, "bvo": bvo, "masks": masks,
        })
    return in_maps


def run(inputs, trace=False):
    from concourse.bass_utils import run_bass_kernel_spmd

    nc = _get_nc()
    in_maps = _make_in_maps(**inputs)
    res = run_bass_kernel_spmd(nc, in_maps, core_ids=list(range(NCORES)),
                               trace=trace)
    out = np.zeros((B, S, D), dtype=np.float32)
    for c in range(NCORES):
        b, g = c // 2, c % 2
        out[b][_perm(g)] = np.asarray(res.results[c]["out"])
    return out, res


def kernel(**inputs):
    out, _ = run(inputs, trace=False)
    return out


# revision 31
# speedup vs baseline: 1.0192x; 1.0192x over previous
"""Causal multi-head attention (B=4, S=2048, H=16, Dh=64) on 8 TRN2 NeuronCores.

Sharding: core c = (batch b=c//2, parity g=c%2). Each core handles the
interleaved 64-row query blocks {64*(2i+g) : i=0..15} of its batch (zig-zag
causal load balancing -> identical SPMD program + equal work on all cores),
with the full K/V of that batch. No collectives needed; host concatenates.

Compute dtype: bf16 matmul inputs, fp32 PSUM accumulation, fp32 output.

Device pipeline per core (single fused Tile program):
  - all input DMAs stream from t=0 (stage pools opened together so SBUF
    regions are disjoint -> no WAR gating of later loads);
  - V proj (token-major, interleaved ones column per head = softmax
    denominator trick); token-tiles 8..15 are emitted as PE filler inside
    head-pair 0's attention so ACT starts exp work early;
  - per head-pair hp: KT/QT proj groups for hp+1 are woven between
    attention blocks of hp as dependency-free PE filler (keeps the
    TensorEngine dense so HAM stays un-throttled);
  - attention per (hp, q-half): scores S^T[k,q] with both heads row-tiled
    concurrently in one PSUM tile -> exp on ScalarE (no max subtraction:
    logits ~N(0,1); masked lanes underflow to exactly 0 like the
    reference) -> causal mask multiply on the diagonal 64-col block ->
    AV accumulation (denominator lands in PSUM row 64 via the ones
    column) -> deferred normalization: fast reciprocal + ones-matmul
    partition broadcast + DVE multiply;
  - WO projection with bias via ones-row K=1 matmul, fp32 out.
"""

import numpy as np
import ml_dtypes

B = 4
S = 2048
D = 1024
H = 16
DH = 64
NCORES = 8
NSLOT = 16  # 64-row query blocks per core
QB = 64  # query block rows
P = 128

_CACHE = {}


def _build():
    import concourse.mybir as mybir
    import concourse.tile as tile
    from concourse import bacc

    dt = mybir.dt
    BF = dt.bfloat16
    F32 = dt.float32
    AF = mybir.ActivationFunctionType

    nc = bacc.Bacc("TRN2", target_bir_lowering=False, debug=False,
                   num_devices=NCORES)

    qT_d = nc.declare_dram_parameter("qT", [D, 1024], BF, isOutput=False)
    kT_d = nc.declare_dram_parameter("kT", [D, S], BF, isOutput=False)
    vT_d = nc.declare_dram_parameter("vT", [D, S], BF, isOutput=False)
    wq_d = nc.declare_dram_parameter("wq", [D, D], BF, isOutput=False)
    wk_d = nc.declare_dram_parameter("wk", [D, D], BF, isOutput=False)
    wv_d = nc.declare_dram_parameter("wv", [D, D], BF, isOutput=False)
    wo_d = nc.declare_dram_parameter("wo", [D, D], BF, isOutput=False)
    bq_d = nc.declare_dram_parameter("bq", [P, 8], F32, isOutput=False)
    bk_d = nc.declare_dram_parameter("bk", [P, 8], F32, isOutput=False)
    bvo_d = nc.declare_dram_parameter("bvo", [1, 2 * D], BF, isOutput=False)
    mask_d = nc.declare_dram_parameter("masks", [P, 2 * QB], BF, isOutput=False)
    out_d = nc.declare_dram_parameter("out", [1024, D], F32, isOutput=True)

    with tile.TileContext(nc) as tc:
        with tc.tile_pool(name="persist", bufs=1) as pp:
            # ---- persistent SBUF tensors (~101 KB/partition) ----
            # V augmented: per token-tile t, head h at cols [65h, 65h+64) plus
            # ones at col 65h+64 (softmax denominator via matmul).
            # per head: cols 0..63 = V, col 64 = ones (denominator row 64)
            Vaug_sb = [pp.tile([P, 65 * H], BF, name=f"Vaug{t}", tag=f"Vaug{t}")
                       for t in range(S // P)]
            OT_sb = [pp.tile([P, 1024], BF, name=f"OTsb{f}", tag=f"OTsb{f}")
                     for f in range(8)]
            ones_sb = pp.tile([1, P], BF, name="ones", tag="ones")
            onesf_sb = pp.tile([1, P], F32, name="onesf", tag="onesf")
            mask_sb = pp.tile([P, 2 * QB], BF, name="masksb", tag="masksb")
            bq_sb = pp.tile([P, 8], F32, name="bqsb", tag="bqsb")
            bk_sb = pp.tile([P, 8], F32, name="bksb", tag="bksb")
            bvo_sb = pp.tile([1, 2 * D], BF, name="bvosb", tag="bvosb")

            nc.vector.memset(ones_sb[:], 1.0)
            nc.vector.memset(onesf_sb[:], 1.0)
            nc.sync.dma_start(mask_sb[:], mask_d[:, :])
            nc.sync.dma_start(bq_sb[:], bq_d[:, :])
            nc.sync.dma_start(bk_sb[:], bk_d[:, :])
            nc.sync.dma_start(bvo_sb[:], bvo_d[:, :])

            # Attention-stage pools open BEFORE the V phase so their SBUF is
            # disjoint from vstage: the kT/qT input DMAs then stream at t=0
            # instead of WAR-waiting for V-proj to release its buffers.
            astack = (
                tc.tile_pool(name="astage", bufs=1),
                tc.tile_pool(name="ktpool", bufs=2),
                tc.tile_pool(name="qtpool", bufs=2),
                tc.tile_pool(name="ppool", bufs=4),
                tc.tile_pool(name="npool", bufs=1),
                tc.tile_pool(name="vstage", bufs=1),
            )
            asp, ktp, qtp, ppool, npool, vsp = [
                p.__enter__() for p in astack]

            wv_sb = [vsp.tile([P, D], BF, name=f"wv{i}", tag=f"wv{i}")
                     for i in range(8)]
            vT_sb = [vsp.tile([P, S], BF, name=f"vT{i}", tag=f"vT{i}")
                     for i in range(8)]
            for i in range(8):
                nc.sync.dma_start(wv_sb[i][:], wv_d[P * i:P * i + P, :])
                nc.sync.dma_start(vT_sb[i][:], vT_d[P * i:P * i + P, :])

            # ========= interleaved: KT proj + QT proj + attention ==========
            # Projection matmul groups for head-pair hp+1 are woven between
            # attention blocks of hp as dependency-free PE filler, so the
            # TensorEngine never idles long enough for HAM to re-throttle.
            F32R = dt.float32r
            with (
                # PSUM: "spair" [P,1024] (2 banks) x 2 + "oe"/"oo" [65,512]
                # (1 bank) x 1 each + "sps" [P,512] x 2 = 8 banks total.
                tc.tile_pool(name="spsum", bufs=2, space="PSUM") as sps,
                tc.tile_pool(name="opsum", bufs=1, space="PSUM") as ops,
            ):
                wk_sb = [asp.tile([P, D], BF, name=f"wk{i}", tag=f"wk{i}")
                         for i in range(8)]
                kT_sb = [asp.tile([P, S], BF, name=f"kT{i}", tag=f"kT{i}")
                         for i in range(8)]
                wq_sb = [asp.tile([P, D], BF, name=f"wq{i}", tag=f"wq{i}")
                         for i in range(8)]
                qT_sb = [asp.tile([P, 1024], BF, name=f"qT{i}", tag=f"qT{i}")
                         for i in range(8)]
                for i in range(8):
                    nc.sync.dma_start(wk_sb[i][:], wk_d[P * i:P * i + P, :])
                    nc.sync.dma_start(kT_sb[i][:], kT_d[P * i:P * i + P, :])
                for i in range(8):
                    nc.sync.dma_start(wq_sb[i][:], wq_d[P * i:P * i + P, :])
                    nc.sync.dma_start(qT_sb[i][:], qT_d[P * i:P * i + P, :])

                def _v_group(t, half):
                    if half == 0:
                        nc.vector.memset(Vaug_sb[t][:], 1.0)
                    ps = sps.tile([P, 512], F32, name="sps", tag="sps")
                    for fi in range(8):
                        nc.tensor.matmul(
                            ps[:],
                            lhsT=vT_sb[fi][:, P * t:P * t + P],
                            rhs=wv_sb[fi][:, 512 * half:512 * half + 512],
                            start=(fi == 0), stop=False)
                    nc.tensor.matmul(
                        ps[:],
                        lhsT=ones_sb[0:1, 0:P],
                        rhs=bvo_sb[0:1, 512 * half:512 * half + 512],
                        start=False, stop=True)
                    src_ap = ps[:].rearrange("p (h w) -> p h w", w=DH)
                    dst = Vaug_sb[t][:].rearrange(
                        "p (h w) -> p h w",
                        w=65)[:, 8 * half:8 * half + 8, 0:DH]
                    nc.vector.tensor_copy(dst, src_ap)

                KT_ts = {}
                QT_ts = {}

                def _alloc_proj(php):
                    KT_ts[php] = ktp.tile([P, S], BF, name="KTrot",
                                          tag="KTrot")
                    QT_ts[php] = qtp.tile([P, 1024], BF, name="QTrot",
                                          tag="QTrot")

                def _kt_group(php, nck):
                    ps = sps.tile([P, 512], F32, name="sps", tag="sps")
                    for fi in range(8):
                        nc.tensor.matmul(
                            ps[:, 0:512],
                            lhsT=wk_sb[fi][:, P * php:P * php + P],
                            rhs=kT_sb[fi][:, 512 * nck:512 * nck + 512],
                            start=(fi == 0), stop=(fi == 7))
                    nc.vector.tensor_scalar(
                        out=KT_ts[php][:, 512 * nck:512 * nck + 512],
                        in0=ps[:, 0:512], scalar1=bk_sb[:, php:php + 1],
                        scalar2=None, op0=mybir.AluOpType.add)

                def _qt_group(php, nck):
                    ps = sps.tile([P, 512], F32, name="sps", tag="sps")
                    for fi in range(8):
                        nc.tensor.matmul(
                            ps[:, 0:512],
                            lhsT=wq_sb[fi][:, P * php:P * php + P],
                            rhs=qT_sb[fi][:, 512 * nck:512 * nck + 512],
                            start=(fi == 0), stop=(fi == 7))
                    nc.vector.tensor_scalar(
                        out=QT_ts[php][:, 512 * nck:512 * nck + 512],
                        in0=ps[:, 0:512], scalar1=bq_sb[:, php:php + 1],
                        scalar2=None, op0=mybir.AluOpType.add)

                pend = []  # (hp, o_psum, head_parity) awaiting normalization

                def _normalize():
                    nhp, nqh, o_ps, h = pend.pop(0)
                    d_sb = npool.tile([1, 512], F32, name="dsb", tag="dsb")
                    nc.vector.tensor_copy(d_sb[:], o_ps[64:65, 0:512])
                    r_sb = npool.tile([1, 512], F32, name="rsb", tag="rsb")
                    with nc.allow_low_precision(
                            reason="recip feeds bf16 output"):
                        nc.vector.reciprocal_approx_fast(r_sb[:], d_sb[:])
                    rb_sb = npool.tile([1, 512], BF, name="rbsb", tag="rbsb")
                    nc.vector.tensor_copy(rb_sb[:], r_sb[:])
                    b_ps = sps.tile([P, 512], F32, name="sps", tag="sps")
                    nc.tensor.matmul(
                        b_ps[0:64, 0:512],
                        lhsT=ones_sb[0:1, 0:64],
                        rhs=rb_sb[0:1, 0:512],
                        start=True, stop=True)
                    b_sb = npool.tile([64, 512], BF, name="bsb", tag="bsb")
                    nc.vector.tensor_copy(b_sb[:], b_ps[0:64, 0:512])
                    nc.vector.tensor_mul(
                        OT_sb[nhp][64 * h:64 * h + 64,
                                   512 * nqh:512 * nqh + 512],
                        o_ps[0:64, 0:512], b_sb[:])

                for t in range(8):
                    for half in range(2):
                        _v_group(t, half)
                _alloc_proj(0)
                for nck in range(4):
                    _kt_group(0, nck)
                for nck in range(2):
                    _qt_group(0, nck)

                for hp in range(8):  # head pair = fo-tile index
                    KT_t = KT_ts[hp]
                    QT_t = QT_ts[hp]
                    fillers = []
                    if hp == 0:
                        fillers += [
                            (lambda t=t, hf=hf: _v_group(t, hf))
                            for t in range(8, 16) for hf in range(2)]
                    if hp + 1 < 8:
                        _alloc_proj(hp + 1)
                        fillers += [
                            (lambda n=n: _kt_group(hp + 1, n))
                            for n in range(4)]
                        fillers += [
                            (lambda n=n: _qt_group(hp + 1, n))
                            for n in range(2)]

                    # --- attention for head pair hp, one q-half at a time ---
                    for qh in range(2):
                        q0 = 512 * qh
                        # drain pending normalizations before reusing o psum
                        while pend:
                            _normalize()
                        o_e = ops.tile([65, 512], F32, name="oe", tag="oe")
                        o_o = ops.tile([65, 512], F32, name="oo", tag="oo")
                        kts = [kt for kt in range(NSLOT)
                               if QB * kt < q0 + 512]
                        nkts = len(kts)
                        for ki, kt in enumerate(kts):
                            c0 = max(QB * kt, q0)
                            c1 = q0 + 512
                            w = c1 - c0
                            # both heads in one tile at 512-stride so the two
                            # K=64 score matmuls pack into disjoint row groups
                            sp = sps.tile([P, 1024], F32, name="spair",
                                          tag="spair")
                            for h in range(2):
                                nc.tensor.matmul(
                                    sp[:, 512 * h:512 * h + w],
                                    lhsT=KT_t[64 * h:64 * h + 64,
                                              P * kt:P * kt + P],
                                    rhs=QT_t[64 * h:64 * h + 64, c0:c1],
                                    start=True, stop=True)
                            pt = ppool.tile([P, 1024], BF, name="pt",
                                            tag="pt")
                            if w >= 160:  # one call incl. the dead gap
                                nc.scalar.activation(pt[:, 0:512 + w],
                                                     sp[:, 0:512 + w], AF.Exp)
                            else:
                                for h in range(2):
                                    nc.scalar.activation(
                                        pt[:, 512 * h:512 * h + w],
                                        sp[:, 512 * h:512 * h + w], AF.Exp)
                            if c0 == QB * kt:  # diagonal 64-col block
                                pm = pt[:, 0:1024].rearrange(
                                    "p (h w) -> p h w", h=2)[:, :, 0:QB]
                                mm = mask_sb[:].rearrange(
                                    "p (h w) -> p h w", h=2)
                                nc.vector.tensor_mul(pm, pm, mm)
                            for h in range(2):
                                o_ps = o_e if h == 0 else o_o
                                nc.tensor.matmul(
                                    o_ps[0:65, c0 - q0:c1 - q0],
                                    lhsT=Vaug_sb[kt][:, 65 * (2 * hp + h):
                                                     65 * (2 * hp + h) + 65],
                                    rhs=pt[:, 512 * h:512 * h + w],
                                    start=(ki == 0), stop=(ki == nkts - 1),
                                    skip_group_check=True)
                            if fillers and (hp == 0 or ki % 4 == 3):
                                fillers.pop(0)()
                        pend.append((hp, qh, o_e, 0))
                        pend.append((hp, qh, o_o, 1))
                    while fillers:
                        fillers.pop(0)()
                while pend:
                    _normalize()

            for p in reversed(astack):
                p.__exit__(None, None, None)

            # ======================= WO projection =========================
            with (
                tc.tile_pool(name="wops", bufs=4, space="PSUM") as wps,
                tc.tile_pool(name="wosb", bufs=1) as wop,
                tc.tile_pool(name="stage", bufs=3) as stp,
            ):
                wo_sb = [wop.tile([P, D], BF, name=f"wo{i}", tag=f"wo{i}")
                         for i in range(8)]
                for i in range(8):
                    nc.sync.dma_start(wo_sb[i][:], wo_d[P * i:P * i + P, :])
                for qt in range(8):
                    st = stp.tile([P, D], F32, name="st", tag="st")
                    for half in range(2):
                        ps = wps.tile([P, 512], F32, name="wps", tag="wps")
                        for f in range(8):
                            nc.tensor.matmul(
                                ps[:],
                                lhsT=OT_sb[f][:, P * qt:P * qt + P],
                                rhs=wo_sb[f][:, 512 * half:512 * half + 512],
                                start=(f == 0), stop=False)
                        nc.tensor.matmul(
                            ps[:],
                            lhsT=ones_sb[0:1, 0:P],
                            rhs=bvo_sb[0:1, D + 512 * half:D + 512 * half + 512],
                            start=False, stop=True)
                        nc.scalar.copy(st[:, 512 * half:512 * half + 512],
                                       ps[:])
                    nc.sync.dma_start(out_d[P * qt:P * qt + P, :], st[:])

    nc.compile()
    return nc


def _get_nc():
    if "nc" not in _CACHE:
        _CACHE["nc"] = _build()
    return _CACHE["nc"]


def _perm(g):
    # local row 64*i+f  <->  global row 64*(2*i+g)+f
    return np.concatenate(
        [QB * (2 * i + g) + np.arange(QB) for i in range(NSLOT)])


def _make_in_maps(q, k, v, WQ, WQ_bias, WK, WK_bias, WV, WV_bias, WO, WO_bias):
    bf = ml_dtypes.bfloat16
    scale = np.float32(1.0 / np.sqrt(DH))
    wq = np.ascontiguousarray(WQ * scale).astype(bf)
    wk = np.ascontiguousarray(WK).astype(bf)
    wv = np.ascontiguousarray(WV).astype(bf)
    wo = np.ascontiguousarray(WO).astype(bf)
    bq = np.ascontiguousarray((WQ_bias * scale).reshape(8, P).T).astype(
        np.float32)
    bk = np.ascontiguousarray(WK_bias.reshape(8, P).T).astype(np.float32)
    bvo = np.ascontiguousarray(
        np.concatenate([WV_bias, WO_bias])[None, :]).astype(bf)

    kT = [np.ascontiguousarray(k[b].T).astype(bf) for b in range(B)]
    vT = [np.ascontiguousarray(v[b].T).astype(bf) for b in range(B)]

    in_maps = []
    for c in range(NCORES):
        b, g = c // 2, c % 2
        perm = _perm(g)
        qT = np.ascontiguousarray(q[b][perm].T).astype(bf)
        # mask for the diagonal 64-col block: [p, 64h+f] = (f >= p - 64*g)
        pgrid = np.arange(P)[:, None]
        fgrid = np.arange(QB)[None, :]
        m = (fgrid >= pgrid - QB * g).astype(np.float32)
        masks = np.ascontiguousarray(np.concatenate([m, m], axis=1)).astype(bf)
        in_maps.append({
            "qT": qT, "kT": kT[b], "vT": vT[b],
            "wq": wq, "wk": wk, "wv": wv, "wo": wo,
            "bq": bq, "bk": bk, "bvo": bvo, "masks": masks,
        })
    return in_maps


def run(inputs, trace=False):
    from concourse.bass_utils import run_bass_kernel_spmd

    nc = _get_nc()
    in_maps = _make_in_maps(**inputs)
    res = run_bass_kernel_spmd(nc, in_maps, core_ids=list(range(NCORES)),
                               trace=trace)
    out = np.zeros((B, S, D), dtype=np.float32)
    for c in range(NCORES):
        b, g = c // 2, c % 2
        out[b][_perm(g)] = np.asarray(res.results[c]["out"])
    return out, res


def kernel(**inputs):
    out, _ = run(inputs, trace=False)
    return out


# revision 33
# speedup vs baseline: 1.1015x; 1.0807x over previous
"""Causal multi-head attention (B=4, S=2048, H=16, Dh=64) on 8 TRN2 NeuronCores.

Sharding: core c = (batch b=c//2, parity g=c%2). Each core handles the
interleaved 64-row query blocks {64*(2i+g) : i=0..15} of its batch (zig-zag
causal load balancing -> identical SPMD program + equal work on all cores),
with the full K/V of that batch. No collectives needed; host concatenates.

Compute dtype: bf16 matmul inputs, fp32 PSUM accumulation, fp32 output.

Device pipeline per core (single fused Tile program):
  - all input DMAs stream from t=0 (stage pools opened together so SBUF
    regions are disjoint -> no WAR gating of later loads);
  - V proj (token-major, interleaved ones column per head = softmax
    denominator trick); token-tiles 8..15 are emitted as PE filler inside
    head-pair 0's attention so ACT starts exp work early;
  - per head-pair hp: KT/QT proj groups for hp+1 are woven between
    attention blocks of hp as dependency-free PE filler (keeps the
    TensorEngine dense so HAM stays un-throttled);
  - attention per (hp, q-half): scores S^T[k,q] with both heads row-tiled
    concurrently in one PSUM tile -> exp on ScalarE (no max subtraction:
    logits ~N(0,1); masked lanes underflow to exactly 0 like the
    reference) -> causal mask multiply on the diagonal 64-col block ->
    AV accumulation (denominator lands in PSUM row 64 via the ones
    column) -> deferred normalization: fast reciprocal + ones-matmul
    partition broadcast + DVE multiply;
  - WO projection with bias via ones-row K=1 matmul, fp32 out.
"""

import numpy as np
import ml_dtypes

B = 4
S = 2048
D = 1024
H = 16
DH = 64
NCORES = 8
NSLOT = 16  # 64-row query blocks per core
QB = 64  # query block rows
P = 128

_CACHE = {}


def _build():
    import concourse.mybir as mybir
    import concourse.tile as tile
    from concourse import bacc

    dt = mybir.dt
    BF = dt.bfloat16
    F32 = dt.float32
    AF = mybir.ActivationFunctionType

    nc = bacc.Bacc("TRN2", target_bir_lowering=False, debug=False,
                   num_devices=NCORES)

    qT_d = nc.declare_dram_parameter("qT", [D, 1024], BF, isOutput=False)
    kT_d = nc.declare_dram_parameter("kT", [D, S], BF, isOutput=False)
    vT_d = nc.declare_dram_parameter("vT", [D, S], BF, isOutput=False)
    wq_d = nc.declare_dram_parameter("wq", [D, D], BF, isOutput=False)
    wk_d = nc.declare_dram_parameter("wk", [D, D], BF, isOutput=False)
    wv_d = nc.declare_dram_parameter("wv", [D, D], BF, isOutput=False)
    wo_d = nc.declare_dram_parameter("wo", [D, D], BF, isOutput=False)
    bq_d = nc.declare_dram_parameter("bq", [P, 8], F32, isOutput=False)
    bk_d = nc.declare_dram_parameter("bk", [P, 8], F32, isOutput=False)
    bvo_d = nc.declare_dram_parameter("bvo", [1, 2 * D], BF, isOutput=False)
    mask_d = nc.declare_dram_parameter("masks", [P, 2 * QB], BF, isOutput=False)
    out_d = nc.declare_dram_parameter("out", [1024, D], F32, isOutput=True)

    with tile.TileContext(nc) as tc:
        with tc.tile_pool(name="persist", bufs=1) as pp:
            # ---- persistent SBUF tensors (~101 KB/partition) ----
            # V augmented: per token-tile t, head h at cols [65h, 65h+64) plus
            # ones at col 65h+64 (softmax denominator via matmul).
            # per head: cols 0..63 = V, col 64 = ones (denominator row 64)
            Vaug_sb = [pp.tile([P, 65 * H], BF, name=f"Vaug{t}", tag=f"Vaug{t}")
                       for t in range(S // P)]
            OT_sb = [pp.tile([P, 1024], BF, name=f"OTsb{f}", tag=f"OTsb{f}")
                     for f in range(8)]
            ones_sb = pp.tile([1, P], BF, name="ones", tag="ones")
            mask_sb = pp.tile([P, 2 * QB], BF, name="masksb", tag="masksb")
            bq_sb = pp.tile([P, 8], F32, name="bqsb", tag="bqsb")
            bk_sb = pp.tile([P, 8], F32, name="bksb", tag="bksb")
            bvo_sb = pp.tile([1, 2 * D], BF, name="bvosb", tag="bvosb")

            nc.vector.memset(ones_sb[:], 1.0)
            nc.sync.dma_start(mask_sb[:], mask_d[:, :])
            nc.sync.dma_start(bq_sb[:], bq_d[:, :])
            nc.sync.dma_start(bk_sb[:], bk_d[:, :])
            nc.sync.dma_start(bvo_sb[:], bvo_d[:, :])

            # Attention-stage pools open BEFORE the V phase so their SBUF is
            # disjoint from vstage: the kT/qT input DMAs then stream at t=0
            # instead of WAR-waiting for V-proj to release its buffers.
            astack = (
                tc.tile_pool(name="astage", bufs=1),
                tc.tile_pool(name="ktpool", bufs=2),
                tc.tile_pool(name="qtpool", bufs=2),
                tc.tile_pool(name="ppool", bufs=3),
                tc.tile_pool(name="npool", bufs=1),
                tc.tile_pool(name="vstage", bufs=1),
            )
            asp, ktp, qtp, ppool, npool, vsp = [
                p.__enter__() for p in astack]

            wv_sb = [vsp.tile([P, D], BF, name=f"wv{i}", tag=f"wv{i}")
                     for i in range(8)]
            vT_sb = [vsp.tile([P, S], BF, name=f"vT{i}", tag=f"vT{i}")
                     for i in range(8)]
            for i in range(8):
                nc.sync.dma_start(wv_sb[i][:], wv_d[P * i:P * i + P, :])
                nc.sync.dma_start(vT_sb[i][:], vT_d[P * i:P * i + P, :])

            # ========= interleaved: KT proj + QT proj + attention ==========
            # Projection matmul groups for head-pair hp+1 are woven between
            # attention blocks of hp as dependency-free PE filler, so the
            # TensorEngine never idles long enough for HAM to re-throttle.
            F32R = dt.float32r
            with (
                # PSUM: "spair" [P,1024] (2 banks) x 2 + "oe"/"oo" [65,512]
                # (1 bank) x 1 each + "sps" [P,512] x 2 = 8 banks total.
                tc.tile_pool(name="spsum", bufs=2, space="PSUM") as sps,
                tc.tile_pool(name="opsum", bufs=1, space="PSUM") as ops,
            ):
                wk_sb = [asp.tile([P, D], BF, name=f"wk{i}", tag=f"wk{i}")
                         for i in range(8)]
                kT_sb = [asp.tile([P, S], BF, name=f"kT{i}", tag=f"kT{i}")
                         for i in range(8)]
                wq_sb = [asp.tile([P, D], BF, name=f"wq{i}", tag=f"wq{i}")
                         for i in range(8)]
                qT_sb = [asp.tile([P, 1024], BF, name=f"qT{i}", tag=f"qT{i}")
                         for i in range(8)]
                for i in range(8):
                    nc.sync.dma_start(wk_sb[i][:], wk_d[P * i:P * i + P, :])
                    nc.sync.dma_start(kT_sb[i][:], kT_d[P * i:P * i + P, :])
                for i in range(8):
                    nc.sync.dma_start(wq_sb[i][:], wq_d[P * i:P * i + P, :])
                    nc.sync.dma_start(qT_sb[i][:], qT_d[P * i:P * i + P, :])

                def _v_group(t, half):
                    if half == 0:
                        nc.vector.memset(Vaug_sb[t][:], 1.0)
                    ps = sps.tile([P, 512], F32, name="sps", tag="sps")
                    for fi in range(8):
                        nc.tensor.matmul(
                            ps[:],
                            lhsT=vT_sb[fi][:, P * t:P * t + P],
                            rhs=wv_sb[fi][:, 512 * half:512 * half + 512],
                            start=(fi == 0), stop=False)
                    nc.tensor.matmul(
                        ps[:],
                        lhsT=ones_sb[0:1, 0:P],
                        rhs=bvo_sb[0:1, 512 * half:512 * half + 512],
                        start=False, stop=True)
                    src_ap = ps[:].rearrange("p (h w) -> p h w", w=DH)
                    dst = Vaug_sb[t][:].rearrange(
                        "p (h w) -> p h w",
                        w=65)[:, 8 * half:8 * half + 8, 0:DH]
                    nc.vector.tensor_copy(dst, src_ap)

                KT_ts = {}
                QT_ts = {}

                def _alloc_proj(php):
                    KT_ts[php] = ktp.tile([P, S], BF, name="KTrot",
                                          tag="KTrot")
                    QT_ts[php] = qtp.tile([P, 1024], BF, name="QTrot",
                                          tag="QTrot")

                def _kt_group(php, nck):
                    ps = sps.tile([P, 512], F32, name="sps", tag="sps")
                    for fi in range(8):
                        nc.tensor.matmul(
                            ps[:, 0:512],
                            lhsT=wk_sb[fi][:, P * php:P * php + P],
                            rhs=kT_sb[fi][:, 512 * nck:512 * nck + 512],
                            start=(fi == 0), stop=(fi == 7))
                    nc.vector.tensor_scalar(
                        out=KT_ts[php][:, 512 * nck:512 * nck + 512],
                        in0=ps[:, 0:512], scalar1=bk_sb[:, php:php + 1],
                        scalar2=None, op0=mybir.AluOpType.add)

                def _qt_group(php, nck):
                    ps = sps.tile([P, 512], F32, name="sps", tag="sps")
                    for fi in range(8):
                        nc.tensor.matmul(
                            ps[:, 0:512],
                            lhsT=wq_sb[fi][:, P * php:P * php + P],
                            rhs=qT_sb[fi][:, 512 * nck:512 * nck + 512],
                            start=(fi == 0), stop=(fi == 7))
                    nc.vector.tensor_scalar(
                        out=QT_ts[php][:, 512 * nck:512 * nck + 512],
                        in0=ps[:, 0:512], scalar1=bq_sb[:, php:php + 1],
                        scalar2=None, op0=mybir.AluOpType.add)

                pend = []  # (hp, o_psum, head_parity) awaiting normalization

                def _normalize():
                    # o_sb is a bf16 SBUF copy -- the PSUM accumulator was
                    # released immediately after AV, so this whole chain is
                    # off the PE critical path
                    nhp, nqh, o_sb, h = pend.pop(0)
                    d_sb = npool.tile([1, 512], F32, name="dsb", tag="dsb")
                    nc.vector.tensor_copy(d_sb[:], o_sb[64:65, 0:512])
                    r_sb = npool.tile([1, 512], F32, name="rsb", tag="rsb")
                    with nc.allow_low_precision(
                            reason="recip feeds bf16 output"):
                        nc.vector.reciprocal_approx_fast(r_sb[:], d_sb[:])
                    rb_sb = npool.tile([1, 512], BF, name="rbsb", tag="rbsb")
                    nc.vector.tensor_copy(rb_sb[:], r_sb[:])
                    b_ps = sps.tile([P, 512], F32, name="sps", tag="sps")
                    nc.tensor.matmul(
                        b_ps[0:64, 0:512],
                        lhsT=ones_sb[0:1, 0:64],
                        rhs=rb_sb[0:1, 0:512],
                        start=True, stop=True)
                    b_sb = npool.tile([64, 512], BF, name="bsb", tag="bsb")
                    nc.vector.tensor_copy(b_sb[:], b_ps[0:64, 0:512])
                    nc.vector.tensor_mul(
                        OT_sb[nhp][64 * h:64 * h + 64,
                                   512 * nqh:512 * nqh + 512],
                        o_sb[0:64, 0:512], b_sb[:])

                for t in range(8):
                    for half in range(2):
                        _v_group(t, half)
                _alloc_proj(0)
                for nck in range(4):
                    _kt_group(0, nck)
                for nck in range(2):
                    _qt_group(0, nck)

                for hp in range(8):  # head pair = fo-tile index
                    KT_t = KT_ts[hp]
                    QT_t = QT_ts[hp]
                    fillers = []
                    if hp == 0:
                        fillers += [
                            (lambda t=t, hf=hf: _v_group(t, hf))
                            for t in range(8, 16) for hf in range(2)]
                    if hp + 1 < 8:
                        _alloc_proj(hp + 1)
                        fillers += [
                            (lambda n=n: _kt_group(hp + 1, n))
                            for n in range(4)]
                        fillers += [
                            (lambda n=n: _qt_group(hp + 1, n))
                            for n in range(2)]

                    # --- attention for head pair hp, one q-half at a time ---
                    for qh in range(2):
                        q0 = 512 * qh
                        # drain pending normalizations before reusing o psum
                        while pend:
                            _normalize()
                        o_e = ops.tile([65, 512], F32, name="oe", tag="oe")
                        o_o = ops.tile([65, 512], F32, name="oo", tag="oo")
                        kts = [kt for kt in range(NSLOT)
                               if QB * kt < q0 + 512]
                        nkts = len(kts)
                        for ki, kt in enumerate(kts):
                            c0 = max(QB * kt, q0)
                            c1 = q0 + 512
                            w = c1 - c0
                            # both heads in one tile at 512-stride so the two
                            # K=64 score matmuls pack into disjoint row groups
                            sp = sps.tile([P, 1024], F32, name="spair",
                                          tag="spair")
                            for h in range(2):
                                nc.tensor.matmul(
                                    sp[:, 512 * h:512 * h + w],
                                    lhsT=KT_t[64 * h:64 * h + 64,
                                              P * kt:P * kt + P],
                                    rhs=QT_t[64 * h:64 * h + 64, c0:c1],
                                    start=True, stop=True)
                            pt = ppool.tile([P, 1024], BF, name="pt",
                                            tag="pt")
                            if w >= 160:  # one call incl. the dead gap
                                nc.scalar.activation(pt[:, 0:512 + w],
                                                     sp[:, 0:512 + w], AF.Exp)
                            else:
                                for h in range(2):
                                    nc.scalar.activation(
                                        pt[:, 512 * h:512 * h + w],
                                        sp[:, 512 * h:512 * h + w], AF.Exp)
                            if c0 == QB * kt:  # diagonal 64-col block
                                pm = pt[:, 0:1024].rearrange(
                                    "p (h w) -> p h w", h=2)[:, :, 0:QB]
                                mm = mask_sb[:].rearrange(
                                    "p (h w) -> p h w", h=2)
                                nc.vector.tensor_mul(pm, pm, mm)
                            for h in range(2):
                                o_ps = o_e if h == 0 else o_o
                                nc.tensor.matmul(
                                    o_ps[0:65, c0 - q0:c1 - q0],
                                    lhsT=Vaug_sb[kt][:, 65 * (2 * hp + h):
                                                     65 * (2 * hp + h) + 65],
                                    rhs=pt[:, 512 * h:512 * h + w],
                                    start=(ki == 0), stop=(ki == nkts - 1),
                                    skip_group_check=True)
                            if fillers and (hp == 0 or ki % 4 == 3):
                                fillers.pop(0)()
                        # evacuate O accumulators to SBUF right away so
                        # the PSUM banks free for the next q-half's AVs
                        ob_e = npool.tile([65, 512], BF, name="osb",
                                          tag="osb", bufs=2)
                        nc.vector.tensor_copy(ob_e[:], o_e[0:65, 0:512])
                        ob_o = npool.tile([65, 512], BF, name="osb",
                                          tag="osb", bufs=2)
                        nc.vector.tensor_copy(ob_o[:], o_o[0:65, 0:512])
                        pend.append((hp, qh, ob_e, 0))
                        pend.append((hp, qh, ob_o, 1))
                    while fillers:
                        fillers.pop(0)()
                while pend:
                    _normalize()

            for p in reversed(astack):
                p.__exit__(None, None, None)

            # ======================= WO projection =========================
            with (
                tc.tile_pool(name="wops", bufs=4, space="PSUM") as wps,
                tc.tile_pool(name="wosb", bufs=1) as wop,
                tc.tile_pool(name="stage", bufs=3) as stp,
            ):
                wo_sb = [wop.tile([P, D], BF, name=f"wo{i}", tag=f"wo{i}")
                         for i in range(8)]
                for i in range(8):
                    nc.sync.dma_start(wo_sb[i][:], wo_d[P * i:P * i + P, :])
                for qt in range(8):
                    st = stp.tile([P, D], F32, name="st", tag="st")
                    for half in range(2):
                        ps = wps.tile([P, 512], F32, name="wps", tag="wps")
                        for f in range(8):
                            nc.tensor.matmul(
                                ps[:],
                                lhsT=OT_sb[f][:, P * qt:P * qt + P],
                                rhs=wo_sb[f][:, 512 * half:512 * half + 512],
                                start=(f == 0), stop=False)
                        nc.tensor.matmul(
                            ps[:],
                            lhsT=ones_sb[0:1, 0:P],
                            rhs=bvo_sb[0:1, D + 512 * half:D + 512 * half + 512],
                            start=False, stop=True)
                        nc.scalar.copy(st[:, 512 * half:512 * half + 512],
                                       ps[:])
                    nc.sync.dma_start(out_d[P * qt:P * qt + P, :], st[:])

    nc.compile()
    return nc


def _get_nc():
    if "nc" not in _CACHE:
        _CACHE["nc"] = _build()
    return _CACHE["nc"]


def _perm(g):
    # local row 64*i+f  <->  global row 64*(2*i+g)+f
    return np.concatenate(
        [QB * (2 * i + g) + np.arange(QB) for i in range(NSLOT)])


def _make_in_maps(q, k, v, WQ, WQ_bias, WK, WK_bias, WV, WV_bias, WO, WO_bias):
    bf = ml_dtypes.bfloat16
    scale = np.float32(1.0 / np.sqrt(DH))
    wq = np.ascontiguousarray(WQ * scale).astype(bf)
    wk = np.ascontiguousarray(WK).astype(bf)
    wv = np.ascontiguousarray(WV).astype(bf)
    wo = np.ascontiguousarray(WO).astype(bf)
    bq = np.ascontiguousarray((WQ_bias * scale).reshape(8, P).T).astype(
        np.float32)
    bk = np.ascontiguousarray(WK_bias.reshape(8, P).T).astype(np.float32)
    bvo = np.ascontiguousarray(
        np.concatenate([WV_bias, WO_bias])[None, :]).astype(bf)

    kT = [np.ascontiguousarray(k[b].T).astype(bf) for b in range(B)]
    vT = [np.ascontiguousarray(v[b].T).astype(bf) for b in range(B)]

    in_maps = []
    for c in range(NCORES):
        b, g = c // 2, c % 2
        perm = _perm(g)
        qT = np.ascontiguousarray(q[b][perm].T).astype(bf)
        # mask for the diagonal 64-col block: [p, 64h+f] = (f >= p - 64*g)
        pgrid = np.arange(P)[:, None]
        fgrid = np.arange(QB)[None, :]
        m = (fgrid >= pgrid - QB * g).astype(np.float32)
        masks = np.ascontiguousarray(np.concatenate([m, m], axis=1)).astype(bf)
        in_maps.append({
            "qT": qT, "kT": kT[b], "vT": vT[b],
            "wq": wq, "wk": wk, "wv": wv, "wo": wo,
            "bq": bq, "bk": bk, "bvo": bvo, "masks": masks,
        })
    return in_maps


def run(inputs, trace=False):
    from concourse.bass_utils import run_bass_kernel_spmd

    nc = _get_nc()
    in_maps = _make_in_maps(**inputs)
    res = run_bass_kernel_spmd(nc, in_maps, core_ids=list(range(NCORES)),
                               trace=trace)
    out = np.zeros((B, S, D), dtype=np.float32)
    for c in range(NCORES):
        b, g = c // 2, c % 2
        out[b][_perm(g)] = np.asarray(res.results[c]["out"])
    return out, res


def kernel(**inputs):
    out, _ = run(inputs, trace=False)
    return out
